# revision 1
# baseline (speedup 1.0000x reference)
"""Trainium2 Bass kernel for windowed attention with relative position bias.

Problem: B=16, N=1168 (12*12 template + 32*32 search), C=256, H=8 heads, Dh=32.
  qkv = x @ w_qkv.T ; per-head attention with rel-pos bias gathered from
  rpb_table via rel_index ; key-mask ; softmax ; out proj + bias.

Sharding: tensor-parallel over heads — core h computes head h for all batches
and its partial output projection; partials are summed on the host (the
all-reduce of the hint) together with b_proj.

Device-side layout trick: scores are computed transposed (keys m on the
partition axis, queries n on the free axis).  Then
  - softmax normalizer comes free as an extra ones-column in the attn@v matmul
  - the key mask folds into v (zeroed rows) instead of the scores
  - the rel-pos bias is applied multiplicatively: p = exp(s)*exp(bias), where
    exp(bias) is materialized once per core into SBUF from the (tiny) per-head
    table using the Toeplitz structure of rel_index (pure strided DMAs).
"""

import sys
import dataclasses

if "/opt/trn_rl_repo" not in sys.path:
    sys.path.insert(0, "/opt/trn_rl_repo")

import ml_dtypes
import numpy as np

import concourse.bass as bass
import concourse.mybir as mybir
import concourse.tile as tile
from concourse import bacc, bass_utils
from concourse.masks import make_identity

dt = mybir.dt

# ---------------------------------------------------------------- constants
B, N, C, H, Dh = 16, 1168, 256, 8, 32
Z, X = 12, 32                      # template / search grid sides
NT, NS = Z * Z, X * X              # 144, 1024
SCALE = float(Dh) ** -0.5
NUM_REL = 23 * 23 + 43 * 43 + 43 * 43 + 63 * 63  # 8196

# zone geometry: zone 0 = template (12x12, base 0), zone 1 = search (32x32, base 144)
ZHW = {0: (Z, Z, 0), 1: (X, X, NT)}

# zone-pair table layout inside the flat [NUM_REL] table input:
# entry (qz, kz): offset, dh-span, dw-span
ZP = {}
_off = 0
for _qz in (0, 1):
    for _kz in (0, 1):
        _hn = ZHW[_qz][0]
        _hm = ZHW[_kz][0]
        _dh = _hn + _hm - 1
        _dw = ZHW[_qz][1] + ZHW[_kz][1] - 1
        ZP[(_qz, _kz)] = (_off, _dh, _dw)
        _off += _dh * _dw
assert _off == NUM_REL

# key-axis tiles: (kz, m0 global key index, hm0, partitions)
M_TILES = [(1, NT + 128 * k, 4 * k, 128) for k in range(8)] + [
    (0, 0, 0, 120),
    (0, 120, 10, 24),
]
# query-axis free-dim chunks for scores (all >=256 so float32r streams 1 cyc/row)
N_CHUNKS = [(0, 512), (512, 368), (880, 288)]
# query-axis tiles for the output projection
N_TILES = [(128 * t, 128) for t in range(9)] + [(1152, 16)]


def _build_nc():
    nc = bacc.Bacc("TRN2", target_bir_lowering=False, debug=False)

    # ---------------- I/O ----------------
    xT = nc.dram_tensor("xT", [B, 2, 128, N], dt.bfloat16, kind="ExternalInput").ap()
    wqkvT = nc.dram_tensor("wqkvT", [2, 128, 96], dt.bfloat16, kind="ExternalInput").ap()
    wprojT = nc.dram_tensor("wprojT", [32, 256], dt.bfloat16, kind="ExternalInput").ap()
    tabs = nc.dram_tensor("tabs", [NUM_REL], dt.float32, kind="ExternalInput").ap()
    maskS_f = nc.dram_tensor("maskS_f", [128, 10, B], dt.float32, kind="ExternalInput").ap()
    out = nc.dram_tensor("out", [B, N, C], dt.float32, kind="ExternalOutput").ap()

    # DRAM scratch
    g_exp = nc.dram_tensor("g_exp", [NUM_REL], dt.bfloat16, kind="Internal").ap()
    E = {}
    for (qz, kz), (off, dhs, dws) in ZP.items():
        Wm = ZHW[kz][1]
        Wn = ZHW[qz][1]
        E[(qz, kz)] = nc.dram_tensor(
            f"E_{qz}{kz}", [dhs, Wm, Wn], dt.bfloat16, kind="Internal"
        ).ap()

    with tile.TileContext(nc) as tc:
        _trace_kernel(tc, xT, wqkvT, wprojT, tabs, maskS_f, out, g_exp, E)

    nc.compile()
    return nc


def _trace_kernel(tc, xT, wqkvT, wprojT, tabs, maskS_f, out, g_exp, E):
    nc = tc.nc
    f32, f32r = dt.float32, dt.float32r
    Exp = mybir.ActivationFunctionType.Exp
    mult, add = mybir.AluOpType.mult, mybir.AluOpType.add

    from contextlib import ExitStack

    ctx = ExitStack()
    const = ctx.enter_context(tc.tile_pool(name="const", bufs=1))
    xpool = ctx.enter_context(tc.tile_pool(name="x", bufs=2))
    qkpool = ctx.enter_context(tc.tile_pool(name="qk", bufs=2))
    ppool = ctx.enter_context(tc.tile_pool(name="p", bufs=22))
    spool = ctx.enter_context(tc.tile_pool(name="s", bufs=2))
    opool = ctx.enter_context(tc.tile_pool(name="o", bufs=2))
    mmps = ctx.enter_context(tc.tile_pool(name="mmps", bufs=3, space="PSUM"))
    ctxps = ctx.enter_context(tc.tile_pool(name="ctxps", bufs=3, space="PSUM"))
    auxps = ctx.enter_context(tc.tile_pool(name="auxps", bufs=2, space="PSUM"))

    # ---------------- one-time setup ----------------
    ident = const.tile([33, 33], f32)
    make_identity(nc, ident[:])
    identr_t = const.tile([33, 33], f32r)
    nc.vector.tensor_copy(identr_t[:], ident[:])
    identr = identr_t[:]
    identb_t = const.tile([33, 33], dt.bfloat16)
    nc.vector.tensor_copy(identb_t[:], ident[:])
    identb = identb_t[:]
    ones33 = const.tile([33, 1], f32)
    nc.vector.memset(ones33[:], 1.0)

    wqkv_sb = const.tile([128, 2, 96], dt.bfloat16)
    nc.sync.dma_start(wqkv_sb[:], wqkvT)
    wproj_sb = const.tile([32, 256], dt.bfloat16)
    nc.sync.dma_start(wproj_sb[:], wprojT)

    # exp the per-head rel-pos table (8196 = 12*683) and round-trip to DRAM
    tabs_sb = const.tile([12, 683], f32)
    nc.sync.dma_start(tabs_sb[:], tabs.rearrange("(a b) -> a b", b=683))
    tabs_e = const.tile([12, 683], dt.bfloat16)
    nc.scalar.activation(tabs_e[:], tabs_sb[:], Exp)
    nc.sync.dma_start(g_exp.rearrange("(a b) -> a b", b=683), tabs_e[:])

    # expand each zone table along w:  E[dh', wm, wn] = g[dh', wn - wm + Wm - 1]
    for (qz, kz), (off, dhs, dws) in ZP.items():
        Wm, Wn = ZHW[kz][1], ZHW[qz][1]
        for wm in range(Wm):
            src = dataclasses.replace(
                g_exp, ap=[[dws, dhs], [1, Wn]], offset=off + (Wm - 1 - wm)
            )
            dst = dataclasses.replace(
                E[(qz, kz)], ap=[[Wm * Wn, dhs], [1, Wn]], offset=wm * Wn
            )
            nc.sync.dma_start(dst, src)

    # broadcast into SBUF-resident ebias[m-part, tile, n]
    ebias = const.tile([128, len(M_TILES), N], dt.bfloat16)
    for ti, (kz, m0, hm0, mcnt) in enumerate(M_TILES):
        Hm, Wm = ZHW[kz][0], ZHW[kz][1]
        nhm = mcnt // Wm
        for dh in range(nhm):
            hm = hm0 + dh
            for qz in (0, 1):
                Hn, Wn, nbase = ZHW[qz]
                dest = ebias[dh * Wm : (dh + 1) * Wm, ti, nbase : nbase + Hn * Wn]
                dest = dest.rearrange("p (a b) -> p a b", b=Wn)
                src = dataclasses.replace(
                    E[(qz, kz)],
                    ap=[[Wn, Wm], [Wm * Wn, Hn], [1, Wn]],
                    offset=(Hm - 1 - hm) * Wm * Wn,
                )
                nc.sync.dma_start(dest, src)

    ebias_h = ebias

    # key mask -> keepT[m-part, tile, b]  (1.0 = keep, 0.0 = masked)
    # mask arrives host-scattered as [m-partition, tile, b] u8; one DMA +
    # one op converting u8 -> 1-x f32.
    keepTu = const.tile([128, len(M_TILES), B], f32)
    nc.sync.dma_start(keepTu[:], maskS_f)
    keepL = const.tile([128, len(M_TILES), B], f32)
    nc.vector.tensor_scalar(keepL[:], keepTu[:], -1.0e30, None, op0=mult)

    # ---------------- per-batch main loop ----------------
    for b in range(B):
        xb_sb = xpool.tile([128, 2, N], dt.bfloat16, tag="xb")
        nc.sync.dma_start(xb_sb[:], xT[b])

        # qkv: [96, n] = wqkvT.T @ xT   (rows 0:32 q, 32:64 k, 64:96 v)
        qT = qkpool.tile([32, N], dt.bfloat16, tag="q")
        kT = qkpool.tile([32, N], dt.bfloat16, tag="k")
        vT = qkpool.tile([33, N], dt.bfloat16, tag="v")
        nc.vector.memset(vT[32:33, :], 1.0)
        for ns, ncnt in N_CHUNKS:
            qkv_ps = mmps.tile([96, 512], f32, tag="mm")
            for c2 in range(2):
                nc.tensor.matmul(
                    qkv_ps[:, :ncnt],
                    wqkv_sb[:, c2, :],
                    xb_sb[:, c2, ns : ns + ncnt],
                    start=(c2 == 0),
                    stop=(c2 == 1),
                )
            nc.vector.tensor_copy(qT[:, ns : ns + ncnt], qkv_ps[0:32, :ncnt])
            nc.vector.tensor_copy(kT[:, ns : ns + ncnt], qkv_ps[32:64, :ncnt])
            nc.vector.tensor_copy(vT[0:32, ns : ns + ncnt], qkv_ps[64:96, :ncnt])

        # v natural + ones column in one transpose: vext[m, 0:32]=v, [:,32]=1
        vext = qkpool.tile([128, len(M_TILES), 33], dt.bfloat16, tag="vext")
        for ti, (kz, m0, hm0, mcnt) in enumerate(M_TILES):
            v_ps = auxps.tile([128, 33], dt.bfloat16, tag="aux")
            nc.tensor.transpose(v_ps[:mcnt, :], vT[:, m0 : m0 + mcnt], identb[:33, :33])
            nc.vector.tensor_copy(vext[:mcnt, ti, :], v_ps[:mcnt, :])

        # attention: scores -> p for all key tiles, then contiguous ctx groups
        pts = []
        for ti, (kz, m0, hm0, mcnt) in enumerate(M_TILES):
            pT = ppool.tile([128, N], dt.bfloat16, tag="p")
            pts.append(pT)
            for ns, ncnt in N_CHUNKS:
                s_ps = mmps.tile([128, 512], f32, tag="mm")
                nc.tensor.matmul(
                    s_ps[:mcnt, :ncnt],
                    kT[:, m0 : m0 + mcnt],
                    qT[:, ns : ns + ncnt],
                    start=True,
                    stop=True,
                )
                nc.scalar.activation(
                    pT[:mcnt, ns : ns + ncnt], s_ps[:mcnt, :ncnt], Exp,
                    bias=keepL[:mcnt, ti, b : b + 1], scale=SCALE,
                )
            nc.vector.tensor_tensor(
                out=pT[:mcnt, :],
                in0=pT[:mcnt, :],
                in1=ebias_h[:mcnt, ti, :],
                op=mult,
            )
        ctx_ps = {}
        for ns, ncnt in N_CHUNKS:
            ctile = ctxps.tile([33, 512], f32, tag="ctx")
            ctx_ps[ns] = ctile
            for ti, (kz, m0, hm0, mcnt) in enumerate(M_TILES):
                nc.tensor.matmul(
                    ctile[:, :ncnt],
                    vext[:mcnt, ti, :],
                    pts[ti][:mcnt, ns : ns + ncnt],
                    start=(ti == 0),
                    stop=(ti == len(M_TILES) - 1),
                )

        # ctxU -> SBUF; rowsums -> reciprocal -> per-n-tile column
        ctx_sb = spool.tile([33, N], dt.bfloat16, tag="ctx_sb")
        ctxs_f = spool.tile([1, N], f32, tag="ctxs_f")
        for ns, ncnt in N_CHUNKS:
            nc.vector.tensor_copy(ctx_sb[:, ns : ns + ncnt], ctx_ps[ns][:, :ncnt])
            nc.vector.tensor_copy(ctxs_f[:, ns : ns + ncnt], ctx_ps[ns][32:33, :ncnt])
        rs_ps = auxps.tile([128, len(N_TILES)], f32, tag="aux")
        for t, (ns, ncnt) in enumerate(N_TILES):
            nc.tensor.transpose(
                rs_ps[:ncnt, t : t + 1],
                ctxs_f[:, ns : ns + ncnt],
                ident[:1, :1],
            )
        rs_raw = spool.tile([128, len(N_TILES)], f32, tag="rs_raw")
        nc.vector.tensor_copy(rs_raw[:, 0:9], rs_ps[:, 0:9])
        nc.vector.tensor_copy(rs_raw[:16, 9:10], rs_ps[:16, 9:10])
        rs_sb = spool.tile([128, len(N_TILES)], f32, tag="rs_sb")
        nc.vector.memset(rs_sb[:], 1.0)
        nc.vector.reciprocal(rs_sb[:, 0:9], rs_raw[:, 0:9])
        nc.vector.reciprocal(rs_sb[:16, 9:10], rs_raw[:16, 9:10])

        # out projection + normalize + store
        o_sb = opool.tile([128, len(N_TILES), 256], f32, tag="o")
        for t, (ns, ncnt) in enumerate(N_TILES):
            pr_ps = auxps.tile([128, 256], f32, tag="aux")
            nc.tensor.matmul(
                pr_ps[:ncnt, :],
                ctx_sb[0:32, ns : ns + ncnt],
                wproj_sb[:],
                start=True,
                stop=True,
            )
            nc.vector.tensor_scalar(
                o_sb[:ncnt, t, :], pr_ps[:ncnt, :], rs_sb[:ncnt, t : t + 1],
                None, op0=mult,
            )
        dst9 = out[b, 0:1152, :].rearrange("(t p) c -> p t c", p=128)
        nc.sync.dma_start(dst9, o_sb[:, 0:9, :])
        nc.sync.dma_start(out[b, 1152:1168, :], o_sb[:16, 9, :])

    ctx.close()


# ---------------------------------------------------------------- host side
_NC_CACHE = {}
LAST_RESULTS = None  # test harness can read exec_time_ns from here


def _perm_tables(rel_index):
    """Flat [NUM_REL] index array: table value j is rel_index at a
    representative (query n, key m) pair realizing that relative offset."""
    perm = np.empty(NUM_REL, np.int64)
    for (qz, kz), (off, dhs, dws) in ZP.items():
        Hn, Wn, nb = ZHW[qz]
        Hm, Wm, mb = ZHW[kz]
        dh = np.arange(dhs)[:, None] - (Hm - 1)   # hn - hm
        dw = np.arange(dws)[None, :] - (Wm - 1)   # wn - wm
        hm = np.maximum(0, -dh)
        hn = dh + hm
        wm = np.maximum(0, -dw)
        wn = dw + wm
        n_rep = nb + hn * Wn + wn                 # [dhs, dws] broadcast
        m_rep = mb + hm * Wm + wm
        perm[off : off + dhs * dws] = rel_index[
            n_rep.astype(np.int64), m_rep.astype(np.int64)
        ].ravel()
    return perm


def kernel(x, mask, w_qkv, w_proj, b_proj, rpb_table, rel_index):
    x = np.asarray(x, np.float32)
    mask = np.asarray(mask)
    w_qkv = np.asarray(w_qkv, np.float32)
    w_proj = np.asarray(w_proj, np.float32)
    b_proj = np.asarray(b_proj, np.float32)
    rpb_table = np.asarray(rpb_table, np.float32)
    rel_index = np.asarray(rel_index)

    if "nc" not in _NC_CACHE:
        _NC_CACHE["nc"] = _build_nc()
    nc = _NC_CACHE["nc"]

    xT = np.ascontiguousarray(x.transpose(0, 2, 1)).reshape(B, 2, 128, N).astype(ml_dtypes.bfloat16)
    mask_u8 = np.ascontiguousarray(mask).view(np.uint8).reshape(B, N)
    maskS = np.zeros((128, len(M_TILES), B), np.float32)
    for ti, (kz, m0, hm0, mcnt) in enumerate(M_TILES):
        maskS[:mcnt, ti, :] = mask_u8[:, m0 : m0 + mcnt].T
    perm = _perm_tables(rel_index)

    in_maps = []
    for h in range(H):
        sl = slice(h * Dh, (h + 1) * Dh)
        w_cat = np.concatenate(
            [w_qkv[0:C][sl], w_qkv[C : 2 * C][sl], w_qkv[2 * C : 3 * C][sl]], axis=0
        )  # [96, 256]
        in_maps.append(
            {
                "xT": xT,
                "wqkvT": np.ascontiguousarray(w_cat.T).reshape(2, 128, 96).astype(ml_dtypes.bfloat16),
                "wprojT": np.ascontiguousarray(w_proj[:, sl].T).astype(ml_dtypes.bfloat16),
                "tabs": np.ascontiguousarray(rpb_table[h][perm]),
                "maskS_f": maskS,
            }
        )

    import os

    trace = bool(int(os.environ.get("KERNEL_TRACE", "0")))
    res = bass_utils.run_bass_kernel_spmd(
        nc, in_maps, core_ids=list(range(H)), trace=trace
    )
    global LAST_RESULTS
    LAST_RESULTS = res

    acc = res.results[0]["out"].astype(np.float32)
    for h in range(1, H):
        acc += res.results[h]["out"]
    acc += b_proj[None, None, :]
    return acc

